# revision 1
# baseline (speedup 1.0000x reference)
"""Trainium2 Bass kernel for nn_Attention (b=4, n=2048, d=1024, 16 heads x 64).

Strategy (8 NeuronCores, zero collectives):
  core i -> batch b = i//2, query-row half h = i%2.
  Each core computes K/V for ALL 2048 positions of its batch (kv projection is
  duplicated across the core pair; ~25% extra PE work buys zero communication),
  and attention + output projection for its 1024 query rows.

  Host-side staging (inside kernel(), not on the device clock):
    - inputs pre-cast to bf16 and pre-laid-out (X pre-transposed to [d, n],
      weights chunked to the exact SBUF layouts the matmuls want)
    - positions permuted so each core's own query rows come first; RoPE
      cos/sin tables are built per-core following the permutation
    - a +/-1 permutation matrix (PermSign) used to compute the RoPE "rotate"
      term as a PE matmul, and a bf16 identity for PE transposes

  Device pipeline per core (all matmuls bf16, fp32 PSUM accumulation):
    1. kT = (Wk^T X^T), qT = (Wq^T X^T) in transposed [chan, pos] layout;
       v in natural [pos, chan] layout with a ones-column interleaved per head
       (so P^T.T @ v65 also produces the softmax row-sums for free).
       RoPE applied in transposed layout: y = cos*x + sin*(PermSign @ x).
    2. Scores S^T[k,q] = kT_h^T @ qT_h per head: two heads run concurrently
       in the PE via 64-row array tiling; all 32 score matmuls of a head-pair
       are batched before the P@V batch so the PE changes tiling mode only
       twice per head-pair. exp on ACT with the 1/sqrt(dh) scale folded in,
       batched over 2 PSUM banks per instruction. P@V with v65 stationary
       accumulates O^T pieces [65, 512] over k-blocks (row 64 = denominator).
    3. DEFERRED NORMALIZATION (v8): the unnormalized O^T pieces are evicted
       to SBUF right after the P@V batch (releasing the PSUM accumulators),
       and the reciprocal/broadcast/normalize chain of head-pair hp is
       emitted after head-pair hp+1's score+exp batch. The PE FIFO therefore
       never blocks on the ~2us DVE reciprocal chain between attention
       units, and the ACT exp chain runs back-to-back across units (the
       profiled baseline lost ~40us of ACT duty to exactly this stall).
       The last head-pair of each q-group normalizes inline so the output
       projection interleave (q-group 0's outproj hides under q-group 1's
       attention) keeps its dependencies.
    4. Output projection straight from O^T, bias added during the fp32
       eviction, DMA out.
"""

import numpy as np
import ml_dtypes

BF16 = ml_dtypes.bfloat16

B, N, D = 4, 2048, 1024
HEADS, DH, ROT = 16, 64, 32
INNER = HEADS * DH          # 1024
NH = N // 2                 # query rows per core
KC = D // 128               # 8 contraction chunks
MC = INNER // 128           # 8 channel chunks (head pairs)
NB = N // 128               # 16 position blocks
SCALE = DH ** -0.5
N_CORES = 8

_CACHE = {}


def _build_nc():
    import concourse.bacc as bacc
    import concourse.mybir as mybir
    import concourse.tile as tile

    dt = mybir.dt
    f32, bf16 = dt.float32, dt.bfloat16
    Alu = mybir.AluOpType
    Act = mybir.ActivationFunctionType

    nc = bacc.Bacc("TRN2", target_bir_lowering=False, debug=False)

    # DRAM parameters (per-core shards; layouts documented in prepare_in_maps)
    xt_d = nc.dram_tensor("xt", [128, KC, N], bf16, kind="ExternalInput")
    wk_d = nc.dram_tensor("wk", [128, MC, KC, 128], bf16, kind="ExternalInput")
    wq_d = nc.dram_tensor("wq", [128, MC, KC, 128], bf16, kind="ExternalInput")
    wv_d = nc.dram_tensor("wv", [128, 2, KC, 512], bf16, kind="ExternalInput")
    wo_d = nc.dram_tensor("wo", [128, MC, D], bf16, kind="ExternalInput")
    bb_d = nc.dram_tensor("bb", [128, D], bf16, kind="ExternalInput")
    cos_d = nc.dram_tensor("cosk", [128, N], bf16, kind="ExternalInput")
    sin_d = nc.dram_tensor("sink", [128, N], bf16, kind="ExternalInput")
    psgn_d = nc.dram_tensor("psgn", [128, 128], bf16, kind="ExternalInput")
    iden_d = nc.dram_tensor("iden", [128, 128], bf16, kind="ExternalInput")
    out_d = nc.dram_tensor("out", [NH, D], f32, kind="ExternalOutput")

    with tile.TileContext(nc) as tc:
        with (
            # ---- resident for the whole kernel ----
            tc.tile_pool(name="const", bufs=1) as constp,
            tc.tile_pool(name="ktr", bufs=1) as ktrp,
            tc.tile_pool(name="qtr", bufs=1) as qtrp,
            tc.tile_pool(name="v65", bufs=1) as v65p,
            tc.tile_pool(name="ot", bufs=1) as otp,
            tc.tile_pool(name="pt", bufs=10) as ptp,
            tc.tile_pool(name="piece", bufs=4) as piecep,
            tc.tile_pool(name="den", bufs=4) as denp,
            tc.tile_pool(name="rvec", bufs=1) as rvp,
            tc.tile_pool(name="bcs", bufs=1) as bcsp,
            # ---- PSUM ----
            tc.tile_pool(name="ps512", bufs=2, space="PSUM") as psp,
            tc.tile_pool(name="pss", bufs=2, space="PSUM") as pssp,
            tc.tile_pool(name="pso", bufs=2, space="PSUM") as psop,
        ):
            cos_sb = constp.tile([128, N], bf16, tag="cos")
            sin_sb = constp.tile([128, N], bf16, tag="sin")
            psgn_sb = constp.tile([128, 128], bf16, tag="psgn")
            ones_pad = constp.tile([128, 128], bf16, tag="ones_pad")
            nc.sync.dma_start(psgn_sb[:], psgn_d.ap())
            nc.vector.memset(ones_pad[:], 0.0)
            nc.vector.memset(ones_pad[0:1, :], 1.0)

            kTr = ktrp.tile([128, MC, N], bf16, tag="kTr")
            qTr = qtrp.tile([128, MC, NH], bf16, tag="qTr")
            v65 = v65p.tile([128, NB, HEADS * 65], bf16, tag="v65")
            oT = otp.tile([128, MC, NH], bf16, tag="oT")
            # reciprocal row for softmax denominators: only partition 0 is
            # ever written; the rest are zeroed once so the broadcast matmul
            # (ones_pad has zeros there) sees no NaN garbage.
            rv = rvp.tile([128, 512], bf16, tag="rv")
            nc.vector.memset(rv[:], 0.0)
            rvf = rvp.tile([1, 512], f32, tag="rvf")

            # ones column per head inside v65 (softmax denominator trick)
            v65_g = v65[:].rearrange("p b (g s) -> p b g s", s=65)
            nc.vector.memset(v65_g[:, :, :, 64:65], 1.0)

            def rope_fin(dst_ap, raw, cos_ap, sin_ap, tmpl):
                """Finish RoPE: dst = cos*raw + sin*(PermSign @ raw).

                Deferred by one j-group relative to raw's eviction, so the
                PE FIFO reaches the PermSign matmul long after the ACT
                eviction landed (inline emission stalled the PE ~0.7us per
                group, 48 times). The z-psum borrows the attention-phase
                pss pool (idle during projections) so the projection
                pipeline keeps both ps512 slots and stays double-buffered."""
                ps_z = pssp.tile([128, 512], f32, tag="pss", name="ps_z")
                nc.tensor.matmul(
                    ps_z[:], psgn_sb[:], raw[:], start=True, stop=True
                )
                zs = tmpl.tile([128, 512], bf16, tag="zs")
                nc.vector.scalar_tensor_tensor(
                    out=zs[:], in0=ps_z[:], scalar=0.0, in1=sin_ap,
                    op0=Alu.bypass, op1=Alu.mult,
                )
                nc.gpsimd.tensor_mul(out=dst_ap, in0=raw[:], in1=cos_ap)
                nc.gpsimd.tensor_add(out=dst_ap, in0=dst_ap, in1=zs[:])

            def rope_step(pending, dst_ap, ps_acc, cos_ap, sin_ap, tmpl):
                """Evict the current group's psum (ACT, which is idle in
                phase 1); finish the PREVIOUS group's RoPE (its raw copy is
                long done). Returns the new pending tuple; the caller
                flushes the last one with rope_flush."""
                raw = tmpl.tile([128, 512], bf16, tag="raw")
                nc.scalar.copy(raw[:], ps_acc)
                if pending is not None:
                    rope_fin(*pending, tmpl)
                return (dst_ap, raw, cos_ap, sin_ap)

            def rope_flush(pending, tmpl):
                if pending is not None:
                    rope_fin(*pending, tmpl)

            # ================= phase 1: projections =================
            with (
                tc.tile_pool(name="xt", bufs=1) as xtp,
                tc.tile_pool(name="wslice", bufs=2) as wsp,
                tc.tile_pool(name="wv", bufs=2) as wvp,
                tc.tile_pool(name="tmp", bufs=3) as tmpp,
            ):
                xt = xtp.tile([128, KC, N], bf16, tag="xt")
                wk_first = wsp.tile([128, KC, 128], bf16, tag="wk_m")
                nc.sync.dma_start(wk_first[:], wk_d.ap()[:, 0])
                for kc in range(KC):
                    nc.sync.dma_start(xt[:, kc], xt_d.ap()[:, kc])
                nc.sync.dma_start(cos_sb[:], cos_d.ap())
                nc.sync.dma_start(sin_sb[:], sin_d.ap())

                # --- kT projection + RoPE (deferred by one j-group) ---
                pend = None
                for m in range(MC):
                    if m == 0:
                        wk_m = wk_first
                    else:
                        wk_m = wsp.tile([128, KC, 128], bf16, tag="wk_m")
                        nc.sync.dma_start(wk_m[:], wk_d.ap()[:, m])
                    for j in range(N // 512):
                        ps = psp.tile([128, 512], f32, tag="ps512")
                        for kc in range(KC):
                            nc.tensor.matmul(
                                ps[:],
                                wk_m[:, kc],
                                xt[:, kc, j * 512:(j + 1) * 512],
                                start=(kc == 0),
                                stop=(kc == KC - 1),
                            )
                        sl = slice(j * 512, (j + 1) * 512)
                        pend = rope_step(pend, kTr[:, m, sl], ps[:],
                                         cos_sb[:, sl], sin_sb[:, sl], tmpp)
                rope_flush(pend, tmpp)

                # --- v projection (natural layout, 65-stride per head) ---
                for vc in range(2):
                    wv_vc = wvp.tile([128, KC, 512], bf16, tag="wv_vc")
                    nc.sync.dma_start(wv_vc[:], wv_d.ap()[:, vc])
                    for nb in range(NB):
                        ps = psp.tile([128, 512], f32, tag="ps512")
                        for kc in range(KC):
                            nc.tensor.matmul(
                                ps[:],
                                xt[:, kc, nb * 128:(nb + 1) * 128],
                                wv_vc[:, kc],
                                start=(kc == 0),
                                stop=(kc == KC - 1),
                            )
                        dst = v65_g[:, nb, vc * 8:(vc + 1) * 8, 0:64]
                        src = ps[:].rearrange("p (g s) -> p g s", s=64)
                        nc.scalar.copy(dst, src)

                # --- qT projection + RoPE (deferred by one j-group) ---
                pend = None
                for m in range(MC):
                    wq_m = wsp.tile([128, KC, 128], bf16, tag="wq_m")
                    nc.sync.dma_start(wq_m[:], wq_d.ap()[:, m])
                    for j in range(NH // 512):
                        ps = psp.tile([128, 512], f32, tag="ps512")
                        for kc in range(KC):
                            nc.tensor.matmul(
                                ps[:],
                                wq_m[:, kc],
                                xt[:, kc, j * 512:(j + 1) * 512],
                                start=(kc == 0),
                                stop=(kc == KC - 1),
                            )
                        sl = slice(j * 512, (j + 1) * 512)
                        pend = rope_step(pend, qTr[:, m, sl], ps[:],
                                         cos_sb[:, sl], sin_sb[:, sl], tmpp)
                rope_flush(pend, tmpp)

            # ============ phase 2: attention, phase 3: out proj ============
            with (
                tc.tile_pool(name="wo", bufs=1) as wop,
                tc.tile_pool(name="bbp", bufs=1) as bbp,
                tc.tile_pool(name="outf", bufs=3) as outfp,
            ):
                wo_sb = wop.tile([128, MC, D], bf16, tag="wo")
                nc.sync.dma_start(wo_sb[:], wo_d.ap())
                bb_sb = bbp.tile([128, D], bf16, tag="bb")
                nc.sync.dma_start(bb_sb[:], bb_d.ap())

                def emit_outproj(nb, dc):
                    ps = psp.tile([128, 512], f32, tag="ps512", name="ps_op")
                    for ic in range(MC):
                        nc.tensor.matmul(
                            ps[:],
                            oT[:, ic, nb * 128:(nb + 1) * 128],
                            wo_sb[:, ic, dc * 512:(dc + 1) * 512],
                            start=(ic == 0),
                            stop=(ic == MC - 1),
                        )
                    outf = outfp.tile([128, 512], f32, tag="outf", name="outf")
                    nc.vector.tensor_tensor(
                        out=outf[:], in0=ps[:],
                        in1=bb_sb[:, dc * 512:(dc + 1) * 512],
                        op=Alu.add,
                    )
                    nc.sync.dma_start(
                        out_d.ap()[nb * 128:(nb + 1) * 128,
                                   dc * 512:(dc + 1) * 512],
                        outf[:],
                    )

                def do_norm(hp, qg, pieces, dens):
                    """oT[ch, q] = piece[ch, q] * (1/den[q]); the den row is
                    broadcast across partitions via the ones_pad matmul."""
                    qsl = slice(qg * 512, (qg + 1) * 512)
                    for h in range(2):
                        hg = 2 * hp + h
                        ic, ph = hg // 2, (hg % 2) * 64
                        nc.vector.reciprocal_approx_fast(
                            rvf[:], dens[h][:]
                        )
                        nc.vector.tensor_copy(rv[0:1, :], rvf[:])
                        bc = psp.tile([128, 512], f32, tag="ps512")
                        nc.tensor.matmul(
                            bc[:], ones_pad[:], rv[:],
                            start=True, stop=True,
                        )
                        bcs = bcsp.tile([64, 512], bf16, tag="bcs")
                        nc.vector.tensor_copy(bcs[:], bc[0:64, :])
                        nc.vector.scalar_tensor_tensor(
                            out=oT[ph:ph + 64, ic, qsl],
                            in0=pieces[h][0:64, :], scalar=0.0, in1=bcs[:],
                            op0=Alu.bypass, op1=Alu.mult,
                        )

                pending = None
                for qg in range(NH // 512):
                    qsl = slice(qg * 512, (qg + 1) * 512)
                    for hp in range(MC):
                        if qg == 1:
                            # fill ACT-bound gaps with the previous q-group's
                            # output projection
                            emit_outproj(hp // 2, hp % 2)
                        # O^T pieces [65, 512]: rows 0:64 = head channels,
                        # row 64 = softmax denominator (ones column of v65)
                        ps_o = [
                            psop.tile([65, 512], f32, tag="pso", name="ps_o")
                            for _ in range(2)
                        ]
                        # all 32 score matmuls in one 64-row-tiled batch
                        # (2 heads concurrent in the PE array), then all PV
                        # matmuls in one full-array batch: 2 tiling-mode
                        # switches per head-pair instead of 32.
                        pts = []
                        for kb in range(NB):
                            ksl = slice(kb * 128, (kb + 1) * 128)
                            ps_s = pssp.tile([128, 1024], f32, tag="pss")
                            for h in range(2):
                                pr = slice(h * 64, (h + 1) * 64)
                                nc.tensor.matmul(
                                    ps_s[:, h * 512:(h + 1) * 512],
                                    kTr[pr, hp, ksl],
                                    qTr[pr, hp, qsl],
                                    start=True, stop=True,
                                )
                            pt = ptp.tile([128, 1024], bf16, tag="pt")
                            nc.scalar.activation(
                                pt[:], ps_s[:], Act.Exp, scale=SCALE
                            )
                            pts.append(pt)
                        for kb in range(NB):
                            for h in range(2):
                                hg = 2 * hp + h
                                nc.tensor.matmul(
                                    ps_o[h][:],
                                    v65_g[:, kb, hg],
                                    pts[kb][:, h * 512:(h + 1) * 512],
                                    start=(kb == 0),
                                    stop=(kb == NB - 1),
                                )
                        # evict the unnormalized pieces + denominator rows
                        # (den to a partition-0 tile: the DVE reciprocal op
                        # miscomputes on HW when fed other partitions),
                        # releasing the PSUM accumulators; normalization of
                        # this head-pair is deferred until after the NEXT
                        # head-pair's scores, so the PE FIFO never blocks on
                        # the DVE reciprocal chain while the ACT exp stream
                        # is starved
                        pieces = [
                            piecep.tile([64, 512], f32, tag="piece",
                                        name="piece")
                            for _ in range(2)
                        ]
                        dens = [
                            denp.tile([1, 512], f32, tag="den", name="den")
                            for _ in range(2)
                        ]
                        for h in range(2):
                            nc.vector.tensor_copy(dens[h][:],
                                                  ps_o[h][64:65, :])
                            nc.vector.tensor_copy(pieces[h][:],
                                                  ps_o[h][0:64, :])
                        if pending is not None:
                            do_norm(*pending)
                            pending = None
                        if hp == MC - 1:
                            # q-group boundary: normalize inline so the
                            # outproj interleave's inputs are complete
                            do_norm(hp, qg, pieces, dens)
                        else:
                            pending = (hp, qg, pieces, dens)

                for qb in range(4):
                    for dc in range(2):
                        emit_outproj(4 + qb, dc)
    nc.compile()
    return nc


def get_nc():
    if "nc" not in _CACHE:
        _CACHE["nc"] = _build_nc()
    return _CACHE["nc"]


def prepare_in_maps(queries, Wq, Wkv, Wout, bout):
    """Host-side staging: shard + pre-layout + pre-cast (bf16)."""
    queries = np.asarray(queries, dtype=np.float32)
    Wq = np.asarray(Wq, dtype=np.float32)
    Wkv = np.asarray(Wkv, dtype=np.float32)
    Wout = np.asarray(Wout, dtype=np.float32)
    bout = np.asarray(bout, dtype=np.float32)

    def chunkT(W, cols):  # [D, cols] -> [128, cols//128, KC, 128]
        return np.ascontiguousarray(
            W.reshape(KC, 128, cols // 128, 128).transpose(1, 2, 0, 3)
        ).astype(BF16)

    wk = chunkT(Wkv[:, :INNER], INNER)
    wq = chunkT(Wq, INNER)
    wv = np.ascontiguousarray(
        Wkv[:, INNER:].reshape(KC, 128, 2, 512).transpose(1, 2, 0, 3)
    ).astype(BF16)
    wo = np.ascontiguousarray(
        Wout.reshape(MC, 128, D).transpose(1, 0, 2)
    ).astype(BF16)
    bb = np.ascontiguousarray(np.broadcast_to(bout, (128, D))).astype(BF16)

    psgn = np.zeros((128, 128), np.float32)
    for base in (0, 64):
        for i in range(ROT // 2):
            psgn[base + 2 * i + 1, base + 2 * i] = -1.0
            psgn[base + 2 * i, base + 2 * i + 1] = 1.0
    psgn = psgn.astype(BF16)
    iden = np.eye(128, dtype=np.float32).astype(BF16)

    inv_freq = (10000.0 ** (-np.arange(0, ROT, 2, dtype=np.float32) / ROT))

    in_maps = []
    for core in range(N_CORES):
        b, h = core // 2, core % 2
        order = np.concatenate([
            np.arange(h * NH, (h + 1) * NH),
            np.arange((1 - h) * NH, (2 - h) * NH),
        ])
        xp = queries[b][order]                      # [N, D]
        xt = np.ascontiguousarray(
            xp.T.reshape(KC, 128, N).transpose(1, 0, 2)
        ).astype(BF16)
        pos = order.astype(np.float32)
        ang = pos[None, :] * inv_freq[:, None]      # [16, N]
        c16, s16 = np.cos(ang), np.sin(ang)
        cosk = np.ones((128, N), np.float32)
        sink = np.zeros((128, N), np.float32)
        for base in (0, 64):
            for c in range(ROT):
                cosk[base + c] = c16[c // 2]
                sink[base + c] = s16[c // 2]
        in_maps.append({
            "xt": xt, "wk": wk, "wq": wq, "wv": wv, "wo": wo, "bb": bb,
            "cosk": cosk.astype(BF16), "sink": sink.astype(BF16),
            "psgn": psgn, "iden": iden,
        })
    return in_maps


def gather(results):
    out = np.empty((B, N, D), np.float32)
    for core in range(N_CORES):
        b, h = core // 2, core % 2
        out[b, h * NH:(h + 1) * NH] = results[core]["out"]
    return out


def kernel(queries, Wq, Wkv, Wout, bout):
    from concourse.bass_utils import run_bass_kernel_spmd

    nc = get_nc()
    in_maps = prepare_in_maps(queries, Wq, Wkv, Wout, bout)
    res = run_bass_kernel_spmd(nc, in_maps, core_ids=list(range(N_CORES)))
    return gather(res.results)

